# revision 42
# baseline (speedup 1.0000x reference)
"""Trainium2 Bass kernel for 4-head spatial self-attention.

Computation (per batch b):
    xf = x[b] reshaped [C=256, n=4096]
    q/k/v = Wq/Wk/Wv @ xf            -> [128, n]   (rows = 4 heads x 32 dims)
    S_h   = (q_h^T k_h) * 32^-0.5    -> [n, n] per head
    P     = exp(S)  (softmax without max-subtraction: logits are O(10), safe)
    A_h   = P_h V_h^T / rowsum       -> [n, 32]
    out   = Wout @ A + bout          -> [C, n]

Sharding: 8 cores = 4 batches x 2 query-halves. Each core handles all 4 heads
for one batch and 2048 queries vs all 4096 keys; outputs are disjoint slices.

Perf notes (cost-model driven):
 - All matmul operands are float32r or bf16: 1 PE cycle per output free-row
   (fp32 costs 4).  float32r needs moving-free >= 256, so the small-free
   matmuls (PV, transposes) use bf16; q/k stay f32r for exact logits.
 - S^T is computed with keys on partitions (queries free) so exp(S^T) tiles
   feed PV directly as stationary operands.  The d=32 head contractions pack
   onto PE row strips via tile_position; concurrent strip matmuls must write
   different PSUM banks ([128, 2, 512] st tile, one bank per head).
 - PV is computed TRANSPOSED: A^T[128q, 33] += pt_slice.T @ [V^T | 1].  The
   moving operand is 33 wide (vs 512 the other way round).  Column 32 of the
   rhs is ones, so A^T col 32 accumulates the softmax denominator for free.
 - exp is split 50/50 across ScalarE (true exp, PSUM->SBUF bf16) and DVE
   (Schraudolph exp2: one tensor_scalar mult+add writing int16 bits that
   reinterpret as bf16 ~= 2^y; device rounds to nearest, sigma=0.0435).
 - Host-side (free): x/weights pre-converted to bf16, weights packed into
   two DMA tensors, and x rotated per core so this core's query half sits
   in columns [0, NQ) -- key order is irrelevant to softmax sums, and it
   removes a separate xq input + its DMA.
 - The (pair, J) stream is software-pipelined: PV lags S/exp by PV_LAG steps
   so the in-order PE queue never waits on the exp engines.
 - A^T normalization: DVE copies acc->SBUF + reciprocal of the denominator
   column; the 16 per-(head,qs) scale-multiplies run on otherwise-idle
   GPSIMD (SBUF only).  A^T is transposed back to [hd, q] with full-128
   PE transposes (bf16) and projected with Wout^T stationary.
"""

import numpy as np
import sys

for _p in ("/opt/trn_rl_repo", "/opt/pypackages"):
    if _p not in sys.path:
        sys.path.append(_p)

import concourse.bass as bass
import concourse.tile as tile
from concourse import bacc, mybir
from concourse.tile import add_dep_helper
from concourse.bass_utils import run_bass_kernel_spmd

f32 = mybir.dt.float32
f32r = mybir.dt.float32r
bf16 = mybir.dt.bfloat16
i16 = mybir.dt.int16

B = 4
C = 256
N = 4096          # h*w = 64*64 key positions
NQ = 2048         # queries per core (half batch)
HEADS = 4
DH = 32
INNER = 128
SCALE = DH ** -0.5

QB = 512          # query block (free dim of S^T tiles)
NQB = NQ // QB    # 4
JT = 128          # key tile (partition dim of S^T tiles)
NJT = N // JT     # 32

# Schraudolph exp2 constants for the bf16 bit pattern:
#   bf16_bits(e^(S*SCALE)) ~= round(128*(S*SCALE*log2(e)) + 128*(127-sigma))
EXP2_SIGMA = 0.0435
EXP2_A = 128.0 * SCALE * 1.4426950408889634
EXP2_B = 128.0 * (127.0 - EXP2_SIGMA)

ACT_SHARE = 0.50  # fraction of exp tiles on ScalarE (rest: DVE Schraudolph)
PV_LAG = 10       # steps PV trails S/exp in the software pipeline
K_VT3_ACT = False  # vT3 copies on DVE
K_HEAD = "serial"  # proj phase before the attention stream (engines idle there)
K_FIXED = "act2"   # k/q/v/acc/a copies on ScalarE; balances the DVE exp load
K_BIAS = "act"   # mid-stream bias adds on ScalarE (last qb splits engines)
K_TQ_LAG = 4     # steps between a qb's last PV flush and its transpose/proj
K_TP_LAG = 1     # steps between a pair's last PV and its normalize
# ScalarE helper slice for DVE exp tiles: measured no gain (the helper
# queues behind ScalarE's own tile, so the st slot frees no earlier); keep 0.
K_HELP = 0


def build_nc():
    nc = bacc.Bacc()

    # x is host-rotated per core so this core's query half is cols [0, NQ);
    # key order is irrelevant (softmax sums over all keys).
    x_d = nc.dram_tensor("x", [C, N], bf16, kind="ExternalInput")
    wqkv_d = nc.dram_tensor("wqkv", [C, 3 * INNER], bf16, kind="ExternalInput")
    eyewo_d = nc.dram_tensor("eyewo", [128, 128 + C], bf16, kind="ExternalInput")
    biasT_d = nc.dram_tensor("biasT", [128, 2], f32, kind="ExternalInput")
    out_d = nc.dram_tensor("out", [C, NQ], f32, kind="ExternalOutput")
    import os
    _dbg = os.environ.get("KDBG", "0") == "1"
    if _dbg:
        dbg_k = nc.dram_tensor("dbg_k", [128, N], f32, kind="ExternalOutput")
        dbg_q = nc.dram_tensor("dbg_q", [128, NQ], f32, kind="ExternalOutput")
        dbg_v3 = nc.dram_tensor("dbg_v3", [128, NJT * HEADS * (DH + 1)], f32, kind="ExternalOutput")
        dbg_pt = nc.dram_tensor("dbg_pt", [128, 2 * QB], f32, kind="ExternalOutput")
        dbg_atb = nc.dram_tensor("dbg_atb", [128, 16 * DH], f32, kind="ExternalOutput")
        dbg_acc = nc.dram_tensor("dbg_acc", [128, 8 * (DH + 1)], f32, kind="ExternalOutput")
        dbg_a = nc.dram_tensor("dbg_a", [128, 512], f32, kind="ExternalOutput")

    with tile.TileContext(nc) as tc:
        import contextlib

        ctx = contextlib.ExitStack()
        with ctx:
            big = ctx.enter_context(tc.tile_pool(name="big", bufs=1))
            wkp = ctx.enter_context(tc.tile_pool(name="wkp", bufs=2))
            ptp = ctx.enter_context(tc.tile_pool(name="ptp", bufs=PV_LAG + 2))
            ps = ctx.enter_context(tc.tile_pool(name="ps", bufs=2, space="PSUM"))

            # ---- constants / weights (packed to minimize DMA count) ----
            wqkv_sb = big.tile([128, 2, 3 * INNER], bf16)  # [c_part, cc, (q|k|v)]
            eyewo_sb = big.tile([128, 128 + C], bf16)      # [inner, (eye|woT)]
            bias_sb = big.tile([128, 2], f32)
            wq_sb = wqkv_sb[:, :, 0:INNER]
            wk_sb = wqkv_sb[:, :, INNER:2 * INNER]
            wv_sb = wqkv_sb[:, :, 2 * INNER:3 * INNER]
            eye_sb = eyewo_sb[:, 0:128]
            wo_sb = eyewo_sb[:, 128:128 + C]

            # ---- activation DMA (chunked, interleaved with projections) ----
            x_sb = big.tile([128, 2, N], bf16)    # [c_part, c_chunk, n]

            k_sb = big.tile([128, N], f32r)       # [inner, n]
            q_sb = big.tile([128, NQ], f32r)      # [inner, nq]
            # v^T + ones col: [j0, (jtile, head), 33]; col 32 stays 1.0
            vT3 = big.tile([128, NJT * HEADS, DH + 1], bf16)
            nc.vector.memset(vT3[:, :, DH:DH + 1], 1.0)

            # ---- DMA priority order: wqkv -> x chunk 0 -> eye/wo/bias ->
            # x chunks 1-3.  q/k projections start as soon as chunk 0 lands
            # (queries are cols [0, NQ) of the rotated x). ----
            def emit_x_dma(ch):
                c0 = 2048 * ch
                for cc in range(2):
                    nc.sync.dma_start(
                        out=x_sb[:, cc, c0:c0 + 2048],
                        in_=x_d[128 * cc:128 * (cc + 1), c0:c0 + 2048],
                    )

            for cc in range(2):
                nc.sync.dma_start(out=wqkv_sb[:, cc, :], in_=wqkv_d[128 * cc:128 * (cc + 1), :])
            emit_x_dma(0)
            nc.sync.dma_start(out=eyewo_sb[:], in_=eyewo_d[:])
            nc.sync.dma_start(out=bias_sb[:], in_=biasT_d[:])
            emit_x_dma(1)

            proj_cp = [0]

            def proj_copy(out, in_):
                # alternate proj-phase psum->sbuf copies across Act/DVE so
                # neither engine serializes the head
                proj_cp[0] += 1
                if proj_cp[0] % 2:
                    nc.scalar.copy(out=out, in_=in_)
                else:
                    nc.vector.tensor_copy(out=out, in_=in_)

            def emit_q_tile(t):
                qp = ps.tile([128, 2, 512], f32, tag="st", bufs=3, name="qp")[:, 0, :]
                for cc in range(2):
                    nc.tensor.matmul(
                        out=qp[:],
                        lhsT=wq_sb[:, cc, :],
                        rhs=x_sb[:, cc, 512 * t:512 * (t + 1)],
                        start=(cc == 0), stop=(cc == 1),
                    )
                proj_copy(q_sb[:, 512 * t:512 * (t + 1)], qp[:])

            for t in range(NQ // 512):
                emit_q_tile(t)

            def emit_k_tile(t):
                kp = ps.tile([128, 2, 512], f32, tag="st", bufs=3, name="kp")[:, 0, :]
                for cc in range(2):
                    nc.tensor.matmul(
                        out=kp[:],
                        lhsT=wk_sb[:, cc, :],
                        rhs=x_sb[:, cc, 512 * t:512 * (t + 1)],
                        start=(cc == 0), stop=(cc == 1),
                    )
                proj_copy(k_sb[:, 512 * t:512 * (t + 1)], kp[:])

            def emit_v_tile(t):
                # v^T computed directly: vT[n, hd] = x_chunk.T @ WvT, with the
                # x chunk as the stationary operand (both bf16, 1 cyc/row).
                # Four 128-row n-tiles accumulate into one PSUM bank; only the
                # first matmul into the bank may use start=True (whole-bank
                # has_written clear).
                vp2_t = ps.tile([128, 2, 512], f32, tag="st", bufs=3, name="vp2")
                vp2 = vp2_t[:, 0, :].rearrange("p (a b) -> p a b", b=128)
                for j2 in range(4):
                    j = 4 * t + j2
                    for cc in range(2):
                        nc.tensor.matmul(
                            out=vp2[:, j2:j2 + 1, :],
                            lhsT=x_sb[:, cc, 128 * j:128 * (j + 1)],
                            rhs=wv_sb[:, cc, :],
                            start=(j2 == 0 and cc == 0), stop=(cc == 1),
                            skip_group_check=True,
                        )
                src = vp2[:, 0:4, :].rearrange("p j (h d) -> p (j h) d", d=DH)
                proj_copy(vT3[:, 16 * t:16 * (t + 1), 0:DH], src)

            if K_HEAD == "serial":
                for t in range(8):
                    emit_k_tile(t)
                    emit_v_tile(t)

            # ---- attention: software-pipelined (qb, p, J) stream ----
            steps = [(qb, p, J) for qb in range(NQB) for p in range(2)
                     for J in range(NJT)]
            nsteps = len(steps)
            exp_err = 0.0
            pt_tiles = {}     # step idx -> pt tile
            acc_tiles = {}    # (qb, p) -> acc psum tile
            atb_tiles = {}    # qb -> normalized A^T sbuf tile
            pv_prev = None
            # scheduled tail work: emitted after the S/PV of the given step
            tail_pair = {}    # step idx -> (qb, p) whose PVs just finished
            tail_qb = {}      # step idx -> qb ready for transpose/proj/out

            for i in range(nsteps + PV_LAG + 5):
                # lazy k/v projection work: k tile t at step 2t, v tile t at
                # step 2t+1, next x chunk ahead of need

                # S^T + exp for step i
                if i < nsteps:
                    qb, p, J = steps[i]
                    q0 = QB * qb
                    if (qb, p) not in acc_tiles:
                        acc_t = ps.tile([128, 512], f32, tag="acc", bufs=2, name="acc")
                        acc_tiles[(qb, p)] = acc_t[:, 0:8 * (DH + 1)].rearrange(
                            "p (a b) -> p a b", b=DH + 1
                        )
                    st = ps.tile([128, 2, QB], f32, tag="st", bufs=3, name="st")
                    for hh in range(2):
                        h = 2 * p + hh
                        nc.tensor.matmul(
                            out=st[:, hh, :],
                            lhsT=k_sb[32 * h:32 * (h + 1), JT * J:JT * (J + 1)],
                            rhs=q_sb[32 * h:32 * (h + 1), q0:q0 + QB],
                            start=True, stop=True,
                            tile_position=(32 * h, 0),
                        )
                    pt = ptp.tile([128, 2, QB], bf16, tag="pt", name="pt")
                    pt_tiles[i] = pt
                    dump_pt = _dbg and i == 0
                    exp_err += ACT_SHARE
                    if exp_err >= 1.0:
                        exp_err -= 1.0
                        nc.scalar.activation(
                            out=pt[:], in_=st[:],
                            func=mybir.ActivationFunctionType.Exp,
                            scale=SCALE,
                        )
                    else:
                        nc.vector.tensor_scalar(
                            out=pt[:].bitcast(i16), in0=st[:],
                            scalar1=EXP2_A, scalar2=EXP2_B,
                            op0=mybir.AluOpType.mult,
                            op1=mybir.AluOpType.add,
                        )
                    if dump_pt:
                        dpt = wkp.tile([128, 2 * QB], f32, tag="dbgpt", name="dbgpt")
                        nc.vector.tensor_copy(out=dpt[:], in_=pt.rearrange("p a b -> p (a b)"))
                        nc.sync.dma_start(out=dbg_pt[:], in_=dpt[:])

                # PV for step i - PV_LAG
                j = i - PV_LAG
                if 0 <= j < nsteps:
                    qb, p, J = steps[j]
                    acc = acc_tiles[(qb, p)]
                    pt = pt_tiles.pop(j)
                    for hh in range(2):
                        h = 2 * p + hh
                        for qs in range(4):
                            # start=True resets has_written for the WHOLE psum
                            # bank (values persist), so only the first matmul
                            # into the bank may set it; later groups' J=0
                            # writes land on cleared bits and start fresh.
                            mm = nc.tensor.matmul(
                                out=acc[:, 2 * qs + hh, :],
                                lhsT=pt[:, hh, 128 * qs:128 * (qs + 1)],
                                rhs=vT3[:, HEADS * J + h, :],
                                start=(J == 0 and hh == 0 and qs == 0),
                                stop=(J == NJT - 1),
                                skip_group_check=True,
                            )
                            if pv_prev is not None:
                                add_dep_helper(mm.ins, pv_prev.ins, sync=False,
                                               reason="pv order")
                            pv_prev = mm
                    if J == NJT - 1:
                        tail_pair[i + K_TP_LAG] = (qb, p)
                        if p == 1:
                            tail_qb[i + (2 if qb == NQB - 1 else K_TQ_LAG)] = qb

                # per-pair normalization as soon as a pair's PVs are done:
                # DVE copies acc->SBUF + reciprocal; GPSIMD scales (SBUF only)
                if i in tail_pair:
                    qb, p = tail_pair.pop(i)
                    acc = acc_tiles.pop((qb, p))
                    if qb not in atb_tiles:
                        atb_tiles[qb] = wkp.tile([128, 16, DH], bf16, tag="atb", name="atb")
                    atb = atb_tiles[qb]
                    rcp_sb = wkp.tile([128, 8], f32, tag="rcp", name="rcp")
                    if True:
                        acc_sb = wkp.tile([128, 8, DH + 1], f32, tag="accsb", name="acc_sb")
                        if K_FIXED == "act2":
                            nc.scalar.copy(out=acc_sb[:], in_=acc[:])
                        else:
                            nc.vector.tensor_copy(out=acc_sb[:], in_=acc[:])
                        if _dbg and qb == 0 and p == 0:
                            nc.sync.dma_start(out=dbg_acc[:], in_=acc_sb.rearrange("p a b -> p (a b)"))
                        nc.vector.reciprocal(out=rcp_sb[:], in_=acc_sb[:, :, DH])
                        for qs in range(4):
                            for hh in range(2):
                                nc.gpsimd.tensor_scalar_mul(
                                    out=atb[:, 4 * qs + 2 * p + hh, :],
                                    in0=acc_sb[:, 2 * qs + hh, 0:DH],
                                    scalar1=rcp_sb[:, 2 * qs + hh:2 * qs + hh + 1],
                                )

                # per-qb finish: transpose A^T -> A, project, bias, DMA out
                if i in tail_qb:
                    qb = tail_qb.pop(i)
                    q0 = QB * qb
                    atb = atb_tiles.pop(qb)
                    a_ps_t = ps.tile([128, 2, 512], f32, tag="st", bufs=3, name="a_ps")
                    a_ps = a_ps_t[:, 0, :].bitcast(bf16).rearrange("p (a b) -> p a b", b=128)
                    atb_flat = atb.rearrange("p i d -> p (i d)")
                    for qs in range(4):
                        nc.tensor.transpose(
                            out=a_ps[:, qs:qs + 1, :],
                            in_=atb_flat[:, 128 * qs:128 * (qs + 1)],
                            identity=eye_sb[:],
                        )
                    a_sb = wkp.tile([128, 4, 128], bf16, tag="asb", name="a_sb")
                    last = qb == NQB - 1
                    if K_FIXED == "act2" and not last:
                        nc.scalar.copy(out=a_sb[:], in_=a_ps[:, 0:4, :])
                    else:
                        nc.vector.tensor_copy(out=a_sb[:], in_=a_ps[:, 0:4, :])
                    if _dbg and qb == 0:
                        datb = wkp.tile([128, 16 * DH], f32, tag="dbgatb", name="dbgatb")
                        nc.vector.tensor_copy(out=datb[:], in_=atb.rearrange("p a b -> p (a b)"))
                        nc.sync.dma_start(out=dbg_atb[:], in_=datb[:])
                        da = wkp.tile([128, 512], f32, tag="dbga", name="dbga")
                        nc.vector.tensor_copy(out=da[:], in_=a_sb.rearrange("p a b -> p (a b)"))
                        nc.sync.dma_start(out=dbg_a[:], in_=da[:])
                    a_flat = a_sb.rearrange("p a b -> p (a b)")
                    for cb in range(2):
                        op = ps.tile([128, 2, QB], f32, tag="st", bufs=3, name="op")[:, 0, :]
                        nc.tensor.matmul(
                            out=op[:],
                            lhsT=wo_sb[:, 128 * cb:128 * (cb + 1)],
                            rhs=a_flat[:],
                            start=True, stop=True,
                        )
                        ob = wkp.tile([128, QB], f32, tag="ob", name="ob")
                        on_act = (K_BIAS == "act" or (K_BIAS == "split" and cb == 0))
                        if last:
                            on_act = cb == 0
                        if on_act:
                            nc.scalar.add(out=ob[:], in_=op[:], add=bias_sb[:, cb:cb + 1])
                        else:
                            nc.vector.tensor_scalar_add(out=ob[:], in0=op[:], scalar1=bias_sb[:, cb:cb + 1])
                        # final qb: issue the second output DMA from the
                        # Activation queue so the two descriptor-gens overlap
                        dma_eng = nc.scalar if (last and cb == 1) else nc.sync
                        dma_eng.dma_start(
                            out=out_d[128 * cb:128 * (cb + 1), q0:q0 + QB], in_=ob[:]
                        )

            # flush any remaining tails
            for i in sorted(list(tail_pair) + list(tail_qb)):
                assert False, "tails must be drained inside the loop"

            if _dbg:
                dk = wkp.tile([128, N], f32, tag="dbgk", name="dbgk")
                nc.vector.tensor_copy(out=dk[:], in_=k_sb[:])
                nc.sync.dma_start(out=dbg_k[:], in_=dk[:])
                dq = wkp.tile([128, NQ], f32, tag="dbgq", name="dbgq")
                nc.vector.tensor_copy(out=dq[:], in_=q_sb[:])
                nc.sync.dma_start(out=dbg_q[:], in_=dq[:])
                dv3 = wkp.tile([128, NJT * HEADS * (DH + 1)], f32, tag="dbgv", name="dbgv")
                nc.vector.tensor_copy(out=dv3[:], in_=vT3.rearrange("p a b -> p (a b)"))
                nc.sync.dma_start(out=dbg_v3[:], in_=dv3[:])



    nc.compile()
    return nc


_NC_CACHE = []


def _get_nc():
    if not _NC_CACHE:
        _NC_CACHE.append(build_nc())
    return _NC_CACHE[0]


def _make_in_maps(x, Wq, Wk, Wv, Wout, bout):
    import ml_dtypes

    bfl = ml_dtypes.bfloat16
    xf = np.ascontiguousarray(x.reshape(B, C, N)).astype(bfl)
    wqkv = np.ascontiguousarray(np.concatenate(
        [np.asarray(w, dtype=np.float32).T for w in (Wq, Wk, Wv)], axis=1
    ).astype(bfl))
    eyewo = np.ascontiguousarray(np.concatenate(
        [np.eye(128, dtype=np.float32),
         np.asarray(Wout, dtype=np.float32).T], axis=1
    ).astype(bfl))
    biasT = np.ascontiguousarray(
        np.asarray(bout, dtype=np.float32).reshape(2, 128).T
    )
    in_maps = []
    for core in range(8):
        b, half = core // 2, core % 2
        q0 = half * NQ
        xr = xf[b] if q0 == 0 else np.ascontiguousarray(np.roll(xf[b], -q0, axis=1))
        in_maps.append({
            "x": xr,
            "wqkv": wqkv, "eyewo": eyewo, "biasT": biasT,
        })
    return in_maps


def kernel(x, Wq, Wk, Wv, Wout, bout):
    nc = _get_nc()
    in_maps = _make_in_maps(x, Wq, Wk, Wv, Wout, bout)
    res = run_bass_kernel_spmd(nc, in_maps, core_ids=list(range(8)))
    out = np.empty((B, C, N), dtype=np.float32)
    for core in range(8):
        b, half = core // 2, core % 2
        q0 = half * NQ
        out[b][:, q0:q0 + NQ] = res.results[core]["out"]
    return out.reshape(B, C, 64, 64)


# revision 45
# speedup vs baseline: 1.0041x; 1.0041x over previous
"""Trainium2 Bass kernel for 4-head spatial self-attention.

Computation (per batch b):
    xf = x[b] reshaped [C=256, n=4096]
    q/k/v = Wq/Wk/Wv @ xf            -> [128, n]   (rows = 4 heads x 32 dims)
    S_h   = (q_h^T k_h) * 32^-0.5    -> [n, n] per head
    P     = exp(S)  (softmax without max-subtraction: logits are O(10), safe)
    A_h   = P_h V_h^T / rowsum       -> [n, 32]
    out   = Wout @ A + bout          -> [C, n]

Sharding: 8 cores = 4 batches x 2 query-halves. Each core handles all 4 heads
for one batch and 2048 queries vs all 4096 keys; outputs are disjoint slices.

Perf notes (cost-model driven):
 - All matmul operands are float32r or bf16: 1 PE cycle per output free-row
   (fp32 costs 4).  float32r needs moving-free >= 256, so the small-free
   matmuls (PV, transposes) use bf16; q/k stay f32r for exact logits.
 - S^T is computed with keys on partitions (queries free) so exp(S^T) tiles
   feed PV directly as stationary operands.  The d=32 head contractions pack
   onto PE row strips via tile_position; concurrent strip matmuls must write
   different PSUM banks ([128, 2, 512] st tile, one bank per head).
 - PV is computed TRANSPOSED: A^T[128q, 33] += pt_slice.T @ [V^T | 1].  The
   moving operand is 33 wide (vs 512 the other way round).  Column 32 of the
   rhs is ones, so A^T col 32 accumulates the softmax denominator for free.
 - exp is split 50/50 across ScalarE (true exp, PSUM->SBUF bf16) and DVE
   (Schraudolph exp2: one tensor_scalar mult+add writing int16 bits that
   reinterpret as bf16 ~= 2^y; device rounds to nearest, sigma=0.0435).
 - Host-side (free): x/weights pre-converted to bf16, weights packed into
   two DMA tensors, and x rotated per core so this core's query half sits
   in columns [0, NQ) -- key order is irrelevant to softmax sums, and it
   removes a separate xq input + its DMA.
 - The (pair, J) stream is software-pipelined: PV lags S/exp by PV_LAG steps
   so the in-order PE queue never waits on the exp engines.
 - A^T normalization: DVE copies acc->SBUF + reciprocal of the denominator
   column; the 16 per-(head,qs) scale-multiplies run on otherwise-idle
   GPSIMD (SBUF only).  A^T is transposed back to [hd, q] with full-128
   PE transposes (bf16) and projected with Wout^T stationary.
"""

import numpy as np
import sys

for _p in ("/opt/trn_rl_repo", "/opt/pypackages"):
    if _p not in sys.path:
        sys.path.append(_p)

import concourse.bass as bass
import concourse.tile as tile
from concourse import bacc, mybir
from concourse.tile import add_dep_helper
from concourse.bass_utils import run_bass_kernel_spmd

f32 = mybir.dt.float32
f32r = mybir.dt.float32r
bf16 = mybir.dt.bfloat16
i16 = mybir.dt.int16

B = 4
C = 256
N = 4096          # h*w = 64*64 key positions
NQ = 2048         # queries per core (half batch)
HEADS = 4
DH = 32
INNER = 128
SCALE = DH ** -0.5

QB = 512          # query block (free dim of S^T tiles)
NQB = NQ // QB    # 4
JT = 128          # key tile (partition dim of S^T tiles)
NJT = N // JT     # 32

# Schraudolph exp2 constants for the bf16 bit pattern:
#   bf16_bits(e^(S*SCALE)) ~= round(128*(S*SCALE*log2(e)) + 128*(127-sigma))
EXP2_SIGMA = 0.0435
EXP2_A = 128.0 * SCALE * 1.4426950408889634
EXP2_B = 128.0 * (127.0 - EXP2_SIGMA)

ACT_SHARE = 0.50  # fraction of exp tiles on ScalarE (rest: DVE Schraudolph)
PV_LAG = 10       # steps PV trails S/exp in the software pipeline
K_VT3_ACT = False  # vT3 copies on DVE
K_HEAD = "serial"  # proj phase before the attention stream (engines idle there)
K_FIXED = "act2"   # k/q/v/acc/a copies on ScalarE; balances the DVE exp load
K_BIAS = "act"   # mid-stream bias adds on ScalarE (last qb splits engines)
K_TQ_LAG = 4     # steps between a qb's last PV flush and its transpose/proj
K_TP_LAG = 1     # steps between a pair's last PV and its normalize
# ScalarE helper-slice for DVE exp tiles: PROVEN HARMFUL (+49-67us!) -- a
# second cross-engine reader on the st tile delays the ring slot recycle
# catastrophically.  Keep 0 (the K_HELP=0 path is the plain DVE op).
K_HELP = 0


def build_nc():
    nc = bacc.Bacc()

    # x is host-rotated per core so this core's query half is cols [0, NQ);
    # key order is irrelevant (softmax sums over all keys).
    x_d = nc.dram_tensor("x", [C, N], bf16, kind="ExternalInput")
    wqkv_d = nc.dram_tensor("wqkv", [C, 3 * INNER], bf16, kind="ExternalInput")
    eyewo_d = nc.dram_tensor("eyewo", [128, 128 + C], bf16, kind="ExternalInput")
    biasT_d = nc.dram_tensor("biasT", [128, 2], f32, kind="ExternalInput")
    out_d = nc.dram_tensor("out", [C, NQ], f32, kind="ExternalOutput")
    import os
    _dbg = os.environ.get("KDBG", "0") == "1"
    if _dbg:
        dbg_k = nc.dram_tensor("dbg_k", [128, N], f32, kind="ExternalOutput")
        dbg_q = nc.dram_tensor("dbg_q", [128, NQ], f32, kind="ExternalOutput")
        dbg_v3 = nc.dram_tensor("dbg_v3", [128, NJT * HEADS * (DH + 1)], f32, kind="ExternalOutput")
        dbg_pt = nc.dram_tensor("dbg_pt", [128, 2 * QB], f32, kind="ExternalOutput")
        dbg_atb = nc.dram_tensor("dbg_atb", [128, 16 * DH], f32, kind="ExternalOutput")
        dbg_acc = nc.dram_tensor("dbg_acc", [128, 8 * (DH + 1)], f32, kind="ExternalOutput")
        dbg_a = nc.dram_tensor("dbg_a", [128, 512], f32, kind="ExternalOutput")

    with tile.TileContext(nc) as tc:
        import contextlib

        ctx = contextlib.ExitStack()
        with ctx:
            big = ctx.enter_context(tc.tile_pool(name="big", bufs=1))
            wkp = ctx.enter_context(tc.tile_pool(name="wkp", bufs=2))
            ptp = ctx.enter_context(tc.tile_pool(name="ptp", bufs=PV_LAG + 2))
            ps = ctx.enter_context(tc.tile_pool(name="ps", bufs=2, space="PSUM"))

            # ---- constants / weights (packed to minimize DMA count) ----
            wqkv_sb = big.tile([128, 2, 3 * INNER], bf16)  # [c_part, cc, (q|k|v)]
            eyewo_sb = big.tile([128, 128 + C], bf16)      # [inner, (eye|woT)]
            bias_sb = big.tile([128, 2], f32)
            wq_sb = wqkv_sb[:, :, 0:INNER]
            wk_sb = wqkv_sb[:, :, INNER:2 * INNER]
            wv_sb = wqkv_sb[:, :, 2 * INNER:3 * INNER]
            eye_sb = eyewo_sb[:, 0:128]
            wo_sb = eyewo_sb[:, 128:128 + C]

            # ---- activation DMA (chunked, interleaved with projections) ----
            x_sb = big.tile([128, 2, N], bf16)    # [c_part, c_chunk, n]

            k_sb = big.tile([128, N], f32r)       # [inner, n]
            q_sb = big.tile([128, NQ], f32r)      # [inner, nq]
            # v^T + ones col: [j0, (jtile, head), 33]; col 32 stays 1.0
            vT3 = big.tile([128, NJT * HEADS, DH + 1], bf16)
            nc.vector.memset(vT3[:, :, DH:DH + 1], 1.0)

            # ---- DMA priority order: wqkv -> x chunk 0 -> eye/wo/bias ->
            # x chunks 1-3.  q/k projections start as soon as chunk 0 lands
            # (queries are cols [0, NQ) of the rotated x). ----
            def emit_x_dma(ch):
                c0 = 2048 * ch
                for cc in range(2):
                    nc.sync.dma_start(
                        out=x_sb[:, cc, c0:c0 + 2048],
                        in_=x_d[128 * cc:128 * (cc + 1), c0:c0 + 2048],
                    )

            for cc in range(2):
                nc.sync.dma_start(out=wqkv_sb[:, cc, :], in_=wqkv_d[128 * cc:128 * (cc + 1), :])
            emit_x_dma(0)
            nc.sync.dma_start(out=eyewo_sb[:], in_=eyewo_d[:])
            nc.sync.dma_start(out=bias_sb[:], in_=biasT_d[:])
            emit_x_dma(1)

            proj_cp = [0]

            def proj_copy(out, in_):
                # alternate proj-phase psum->sbuf copies across Act/DVE so
                # neither engine serializes the head
                proj_cp[0] += 1
                if proj_cp[0] % 2:
                    nc.scalar.copy(out=out, in_=in_)
                else:
                    nc.vector.tensor_copy(out=out, in_=in_)

            def emit_q_tile(t):
                qp = ps.tile([128, 2, 512], f32, tag="st", bufs=3, name="qp")[:, 0, :]
                for cc in range(2):
                    nc.tensor.matmul(
                        out=qp[:],
                        lhsT=wq_sb[:, cc, :],
                        rhs=x_sb[:, cc, 512 * t:512 * (t + 1)],
                        start=(cc == 0), stop=(cc == 1),
                    )
                proj_copy(q_sb[:, 512 * t:512 * (t + 1)], qp[:])

            for t in range(NQ // 512):
                emit_q_tile(t)

            def emit_k_tile(t):
                kp = ps.tile([128, 2, 512], f32, tag="st", bufs=3, name="kp")[:, 0, :]
                for cc in range(2):
                    nc.tensor.matmul(
                        out=kp[:],
                        lhsT=wk_sb[:, cc, :],
                        rhs=x_sb[:, cc, 512 * t:512 * (t + 1)],
                        start=(cc == 0), stop=(cc == 1),
                    )
                proj_copy(k_sb[:, 512 * t:512 * (t + 1)], kp[:])

            def emit_v_tile(t):
                # v^T computed directly: vT[n, hd] = x_chunk.T @ WvT, with the
                # x chunk as the stationary operand (both bf16, 1 cyc/row).
                # Four 128-row n-tiles accumulate into one PSUM bank; only the
                # first matmul into the bank may use start=True (whole-bank
                # has_written clear).
                vp2_t = ps.tile([128, 2, 512], f32, tag="st", bufs=3, name="vp2")
                vp2 = vp2_t[:, 0, :].rearrange("p (a b) -> p a b", b=128)
                for j2 in range(4):
                    j = 4 * t + j2
                    for cc in range(2):
                        nc.tensor.matmul(
                            out=vp2[:, j2:j2 + 1, :],
                            lhsT=x_sb[:, cc, 128 * j:128 * (j + 1)],
                            rhs=wv_sb[:, cc, :],
                            start=(j2 == 0 and cc == 0), stop=(cc == 1),
                            skip_group_check=True,
                        )
                src = vp2[:, 0:4, :].rearrange("p j (h d) -> p (j h) d", d=DH)
                proj_copy(vT3[:, 16 * t:16 * (t + 1), 0:DH], src)

            if K_HEAD == "serial":
                for t in range(8):
                    emit_k_tile(t)
                    emit_v_tile(t)

            # ---- attention: software-pipelined (qb, p, J) stream ----
            steps = [(qb, p, J) for qb in range(NQB) for p in range(2)
                     for J in range(NJT)]
            nsteps = len(steps)
            # 0.5 phase => ScalarE takes the first stream tile; aligns the
            # engine alternation with the st-ring parity (-0.8us measured)
            exp_err = 0.5
            pt_tiles = {}     # step idx -> pt tile
            acc_tiles = {}    # (qb, p) -> acc psum tile
            atb_tiles = {}    # qb -> normalized A^T sbuf tile
            pv_prev = None
            # scheduled tail work: emitted after the S/PV of the given step
            tail_pair = {}    # step idx -> (qb, p) whose PVs just finished
            tail_qb = {}      # step idx -> qb ready for transpose/proj/out

            for i in range(nsteps + PV_LAG + 5):
                # lazy k/v projection work: k tile t at step 2t, v tile t at
                # step 2t+1, next x chunk ahead of need

                # S^T + exp for step i
                if i < nsteps:
                    qb, p, J = steps[i]
                    q0 = QB * qb
                    if (qb, p) not in acc_tiles:
                        acc_t = ps.tile([128, 512], f32, tag="acc", bufs=2, name="acc")
                        acc_tiles[(qb, p)] = acc_t[:, 0:8 * (DH + 1)].rearrange(
                            "p (a b) -> p a b", b=DH + 1
                        )
                    st = ps.tile([128, 2, QB], f32, tag="st", bufs=3, name="st")
                    for hh in range(2):
                        h = 2 * p + hh
                        nc.tensor.matmul(
                            out=st[:, hh, :],
                            lhsT=k_sb[32 * h:32 * (h + 1), JT * J:JT * (J + 1)],
                            rhs=q_sb[32 * h:32 * (h + 1), q0:q0 + QB],
                            start=True, stop=True,
                            tile_position=(32 * h, 0),
                        )
                    pt = ptp.tile([128, 2, QB], bf16, tag="pt", name="pt")
                    pt_tiles[i] = pt
                    dump_pt = _dbg and i == 0
                    exp_err += ACT_SHARE
                    if exp_err >= 1.0:
                        exp_err -= 1.0
                        nc.scalar.activation(
                            out=pt[:], in_=st[:],
                            func=mybir.ActivationFunctionType.Exp,
                            scale=SCALE,
                        )
                    else:
                        st_flat = st[:].rearrange("p a b -> p (a b)")
                        pt_flat = pt[:].rearrange("p a b -> p (a b)")
                        nc.vector.tensor_scalar(
                            out=pt_flat[:, 0:1024 - K_HELP].bitcast(i16),
                            in0=st_flat[:, 0:1024 - K_HELP],
                            scalar1=EXP2_A, scalar2=EXP2_B,
                            op0=mybir.AluOpType.mult,
                            op1=mybir.AluOpType.add,
                        )
                        if K_HELP:
                            nc.scalar.activation(
                                out=pt_flat[:, 1024 - K_HELP:],
                                in_=st_flat[:, 1024 - K_HELP:],
                                func=mybir.ActivationFunctionType.Exp,
                                scale=SCALE,
                            )
                    if dump_pt:
                        dpt = wkp.tile([128, 2 * QB], f32, tag="dbgpt", name="dbgpt")
                        nc.vector.tensor_copy(out=dpt[:], in_=pt.rearrange("p a b -> p (a b)"))
                        nc.sync.dma_start(out=dbg_pt[:], in_=dpt[:])

                # PV for step i - PV_LAG
                j = i - PV_LAG
                if 0 <= j < nsteps:
                    qb, p, J = steps[j]
                    acc = acc_tiles[(qb, p)]
                    pt = pt_tiles.pop(j)
                    for hh in range(2):
                        h = 2 * p + hh
                        for qs in range(4):
                            # start=True resets has_written for the WHOLE psum
                            # bank (values persist), so only the first matmul
                            # into the bank may set it; later groups' J=0
                            # writes land on cleared bits and start fresh.
                            mm = nc.tensor.matmul(
                                out=acc[:, 2 * qs + hh, :],
                                lhsT=pt[:, hh, 128 * qs:128 * (qs + 1)],
                                rhs=vT3[:, HEADS * J + h, :],
                                start=(J == 0 and hh == 0 and qs == 0),
                                stop=(J == NJT - 1),
                                skip_group_check=True,
                            )
                            if pv_prev is not None:
                                add_dep_helper(mm.ins, pv_prev.ins, sync=False,
                                               reason="pv order")
                            pv_prev = mm
                    if J == NJT - 1:
                        tail_pair[i + K_TP_LAG] = (qb, p)
                        if p == 1:
                            tail_qb[i + (2 if qb == NQB - 1 else K_TQ_LAG)] = qb

                # per-pair normalization as soon as a pair's PVs are done:
                # DVE copies acc->SBUF + reciprocal; GPSIMD scales (SBUF only)
                if i in tail_pair:
                    qb, p = tail_pair.pop(i)
                    acc = acc_tiles.pop((qb, p))
                    if qb not in atb_tiles:
                        atb_tiles[qb] = wkp.tile([128, 16, DH], bf16, tag="atb", name="atb")
                    atb = atb_tiles[qb]
                    rcp_sb = wkp.tile([128, 8], f32, tag="rcp", name="rcp")
                    if True:
                        acc_sb = wkp.tile([128, 8, DH + 1], f32, tag="accsb", name="acc_sb")
                        if K_FIXED == "act2":
                            nc.scalar.copy(out=acc_sb[:], in_=acc[:])
                        else:
                            nc.vector.tensor_copy(out=acc_sb[:], in_=acc[:])
                        if _dbg and qb == 0 and p == 0:
                            nc.sync.dma_start(out=dbg_acc[:], in_=acc_sb.rearrange("p a b -> p (a b)"))
                        nc.vector.reciprocal(out=rcp_sb[:], in_=acc_sb[:, :, DH])
                        for qs in range(4):
                            for hh in range(2):
                                nc.gpsimd.tensor_scalar_mul(
                                    out=atb[:, 4 * qs + 2 * p + hh, :],
                                    in0=acc_sb[:, 2 * qs + hh, 0:DH],
                                    scalar1=rcp_sb[:, 2 * qs + hh:2 * qs + hh + 1],
                                )

                # per-qb finish: transpose A^T -> A, project, bias, DMA out
                if i in tail_qb:
                    qb = tail_qb.pop(i)
                    q0 = QB * qb
                    atb = atb_tiles.pop(qb)
                    a_ps_t = ps.tile([128, 2, 512], f32, tag="st", bufs=3, name="a_ps")
                    a_ps = a_ps_t[:, 0, :].bitcast(bf16).rearrange("p (a b) -> p a b", b=128)
                    atb_flat = atb.rearrange("p i d -> p (i d)")
                    for qs in range(4):
                        nc.tensor.transpose(
                            out=a_ps[:, qs:qs + 1, :],
                            in_=atb_flat[:, 128 * qs:128 * (qs + 1)],
                            identity=eye_sb[:],
                        )
                    a_sb = wkp.tile([128, 4, 128], bf16, tag="asb", name="a_sb")
                    last = qb == NQB - 1
                    if K_FIXED == "act2" and not last:
                        nc.scalar.copy(out=a_sb[:], in_=a_ps[:, 0:4, :])
                    else:
                        nc.vector.tensor_copy(out=a_sb[:], in_=a_ps[:, 0:4, :])
                    if _dbg and qb == 0:
                        datb = wkp.tile([128, 16 * DH], f32, tag="dbgatb", name="dbgatb")
                        nc.vector.tensor_copy(out=datb[:], in_=atb.rearrange("p a b -> p (a b)"))
                        nc.sync.dma_start(out=dbg_atb[:], in_=datb[:])
                        da = wkp.tile([128, 512], f32, tag="dbga", name="dbga")
                        nc.vector.tensor_copy(out=da[:], in_=a_sb.rearrange("p a b -> p (a b)"))
                        nc.sync.dma_start(out=dbg_a[:], in_=da[:])
                    a_flat = a_sb.rearrange("p a b -> p (a b)")
                    for cb in range(2):
                        op = ps.tile([128, 2, QB], f32, tag="st", bufs=3, name="op")[:, 0, :]
                        nc.tensor.matmul(
                            out=op[:],
                            lhsT=wo_sb[:, 128 * cb:128 * (cb + 1)],
                            rhs=a_flat[:],
                            start=True, stop=True,
                        )
                        ob = wkp.tile([128, QB], f32, tag="ob", name="ob")
                        on_act = (K_BIAS == "act" or (K_BIAS == "split" and cb == 0))
                        if last:
                            on_act = cb == 0
                        if on_act:
                            nc.scalar.add(out=ob[:], in_=op[:], add=bias_sb[:, cb:cb + 1])
                        else:
                            nc.vector.tensor_scalar_add(out=ob[:], in0=op[:], scalar1=bias_sb[:, cb:cb + 1])
                        # final qb: issue the second output DMA from the
                        # Activation queue so the two descriptor-gens overlap
                        dma_eng = nc.scalar if (last and cb == 1) else nc.sync
                        dma_eng.dma_start(
                            out=out_d[128 * cb:128 * (cb + 1), q0:q0 + QB], in_=ob[:]
                        )

            # flush any remaining tails
            for i in sorted(list(tail_pair) + list(tail_qb)):
                assert False, "tails must be drained inside the loop"

            if _dbg:
                dk = wkp.tile([128, N], f32, tag="dbgk", name="dbgk")
                nc.vector.tensor_copy(out=dk[:], in_=k_sb[:])
                nc.sync.dma_start(out=dbg_k[:], in_=dk[:])
                dq = wkp.tile([128, NQ], f32, tag="dbgq", name="dbgq")
                nc.vector.tensor_copy(out=dq[:], in_=q_sb[:])
                nc.sync.dma_start(out=dbg_q[:], in_=dq[:])
                dv3 = wkp.tile([128, NJT * HEADS * (DH + 1)], f32, tag="dbgv", name="dbgv")
                nc.vector.tensor_copy(out=dv3[:], in_=vT3.rearrange("p a b -> p (a b)"))
                nc.sync.dma_start(out=dbg_v3[:], in_=dv3[:])



    nc.compile()
    return nc


_NC_CACHE = []


def _get_nc():
    if not _NC_CACHE:
        _NC_CACHE.append(build_nc())
    return _NC_CACHE[0]


def _make_in_maps(x, Wq, Wk, Wv, Wout, bout):
    import ml_dtypes

    bfl = ml_dtypes.bfloat16
    xf = np.ascontiguousarray(x.reshape(B, C, N)).astype(bfl)
    wqkv = np.ascontiguousarray(np.concatenate(
        [np.asarray(w, dtype=np.float32).T for w in (Wq, Wk, Wv)], axis=1
    ).astype(bfl))
    eyewo = np.ascontiguousarray(np.concatenate(
        [np.eye(128, dtype=np.float32),
         np.asarray(Wout, dtype=np.float32).T], axis=1
    ).astype(bfl))
    biasT = np.ascontiguousarray(
        np.asarray(bout, dtype=np.float32).reshape(2, 128).T
    )
    in_maps = []
    for core in range(8):
        b, half = core // 2, core % 2
        q0 = half * NQ
        xr = xf[b] if q0 == 0 else np.ascontiguousarray(np.roll(xf[b], -q0, axis=1))
        in_maps.append({
            "x": xr,
            "wqkv": wqkv, "eyewo": eyewo, "biasT": biasT,
        })
    return in_maps


def kernel(x, Wq, Wk, Wv, Wout, bout):
    nc = _get_nc()
    in_maps = _make_in_maps(x, Wq, Wk, Wv, Wout, bout)
    res = run_bass_kernel_spmd(nc, in_maps, core_ids=list(range(8)))
    out = np.empty((B, C, N), dtype=np.float32)
    for core in range(8):
        b, half = core // 2, core % 2
        q0 = half * NQ
        out[b][:, q0:q0 + NQ] = res.results[core]["out"]
    return out.reshape(B, C, 64, 64)
